# revision 19
# baseline (speedup 1.0000x reference)
"""Trainium2 Bass kernel for capsule-routing GNN message passing.

Problem: nn_COSAL_33981781246135 (gnn_message_passing).

Strategy (graph/data parallel, per the sharding hint):
  - Targets are sharded contiguously across the 8 cores (2048 targets each).
  - Each core receives its incident edges' neighbor rows pre-gathered on the
    host (x_nb[col_idx] for its edge range), already transposed + cast to bf16.
  - On-device layout is target-major "slot" form: each 128-target tile has its
    targets on partitions and its edges padded to J slots along the free dim.
    Targets are degree-sorted on the host so J is near the tile's mean degree.
    All segment ops (softmax sums, scatter-adds) become free-dim reduces or
    PE identity-matmul PSUM accumulations - no one-hot matmuls, no gathers.
  - All per-(target,capsule) normalizations (1/S softmax denominators, capsule
    l2 norms) are algebraically folded into the next per-edge logit scale, so
    z and u are kept raw in bf16 and never rescaled in memory.
"""

import os
import sys
import time

for _p in ("/opt/trn_rl_repo", os.path.expanduser("~/.axon_site/_ro/trn_rl_repo")):
    if os.path.isdir(_p) and _p not in sys.path:
        sys.path.insert(0, _p)

import numpy as np
import ml_dtypes
from contextlib import ExitStack

import concourse.bass as bass
import concourse.bacc as bacc
import concourse.mybir as mybir
from concourse import tile
from concourse.bass_utils import run_bass_kernel_spmd

BF16 = mybir.dt.bfloat16
F32 = mybir.dt.float32
AX = mybir.AxisListType
ALU = mybir.AluOpType
ACTF = mybir.ActivationFunctionType

NCORES = 8
K = 8          # capsules
DD = 64        # per-capsule dim
D = 512
T = 16384      # targets
NB = 100000
E = 131072
TPC = T // NCORES        # 2048 targets per core
NTILES = TPC // 128      # 16 tiles per core
ROUIT = 3
BETA = 0.5
JC = 6                   # slot-columns per chunk in the routing loop
MASKNEG = -40.0
EPS = 1e-6

bf16 = ml_dtypes.bfloat16


# ----------------------------------------------------------------------------
# Host-side layout construction
# ----------------------------------------------------------------------------

class Layout:
    pass


def build_layout(row_idx, col_idx, ppr):
    """Compute the unified slot layout + per-core input tensors."""
    lay = Layout()
    bounds = np.searchsorted(row_idx, np.arange(NCORES + 1) * TPC).astype(np.int64)
    cores = []
    for c in range(NCORES):
        e0, e1 = int(bounds[c]), int(bounds[c + 1])
        r = row_idx[e0:e1].astype(np.int64) - c * TPC
        deg = np.bincount(r, minlength=TPC)
        order = np.argsort(-deg, kind="stable")
        inv_order = np.empty(TPC, dtype=np.int64)
        inv_order[order] = np.arange(TPC)
        cores.append((e0, e1, r, deg, order, inv_order))

    # Unified per-tile slot count J (max over cores so one program fits all).
    J = []
    for t in range(NTILES):
        m = 1
        for (_, _, _, deg, order, _) in cores:
            m = max(m, int(deg[order[t * 128:(t + 1) * 128]].max()))
        J.append(m)
    lay.J = J
    lay.SJ = int(sum(J))
    lay.NSLOT = 128 * lay.SJ
    lay.coff = np.concatenate([[0], np.cumsum(J)]).astype(np.int64)  # col offsets

    # Map each slot-column to (tile, j) for the builder.
    col2tile = []
    for t in range(NTILES):
        for j in range(J[t]):
            col2tile.append((t, j))
    lay.col2tile = col2tile

    lay.cores = []
    for (e0, e1, r, deg, order, inv_order) in cores:
        ec = e1 - e0
        # Edge -> slot position. Edges are sorted by r, so the rank of an edge
        # within its target is e - start[r[e]].
        starts = np.concatenate([[0], np.cumsum(deg)]).astype(np.int64)
        eloc = np.arange(ec, dtype=np.int64)
        jrank = eloc - starts[r]
        pos = inv_order[r]                       # position in degree-sorted order
        tl = pos // 128
        part = pos % 128
        col = lay.coff[tl] + jrank               # global slot-column
        slot = col * 128 + part                  # flat slot id
        eid = np.full(lay.NSLOT, -1, dtype=np.int64)
        eid[slot] = eloc
        cd = {}
        cd["e0"], cd["e1"] = e0, e1
        cd["order"] = order
        cd["eid"] = eid
        lay.cores.append(cd)
    return lay


def build_core_inputs(lay, c, x_nb, col_idx, ppr):
    cd = lay.cores[c]
    e0, eid = cd["e0"], cd["eid"]
    valid = eid >= 0
    cols = np.where(valid, col_idx[e0:][np.maximum(eid, 0)], 0)
    xg = x_nb[cols]                              # (NSLOT, 512) f32
    xgt = np.ascontiguousarray(xg.T).astype(bf16)  # (512, NSLOT)
    pprs = np.where(valid, ppr[e0:][np.maximum(eid, 0)], MASKNEG).astype(np.float32)
    pprs = np.ascontiguousarray(pprs.reshape(lay.SJ, 128).T)      # (128, SJ)
    maskn = np.where(valid, 0.0, MASKNEG).astype(np.float32)
    maskn = np.ascontiguousarray(maskn.reshape(lay.SJ, 128).T)    # (128, SJ)
    return {"xgt": xgt, "pprs": pprs, "maskn": maskn}


# ----------------------------------------------------------------------------
# Device program
# ----------------------------------------------------------------------------

def chunks_of(J):
    out = []
    c0 = 0
    while c0 < J:
        out.append((c0, min(JC, J - c0)))
        c0 += JC
    return out


def build_program(lay, has_pca_b):
    nc = bacc.Bacc("TRN2", target_bir_lowering=False, debug=False)
    SJ, J, coff = lay.SJ, lay.J, lay.coff

    # DRAM I/O
    xgt_d = nc.dram_tensor("xgt", [512, lay.NSLOT], BF16, kind="ExternalInput")
    pca_w_d = nc.dram_tensor("pca_w", [512, 512], BF16, kind="ExternalInput")
    pprs_d = nc.dram_tensor("pprs", [128, SJ], F32, kind="ExternalInput")
    maskn_d = nc.dram_tensor("maskn", [128, SJ], F32, kind="ExternalInput")
    mlp_w_d = nc.dram_tensor("mlp_w", [512, 40], F32, kind="ExternalInput")
    mlp_b_d = nc.dram_tensor("mlp_b", [1, 40], F32, kind="ExternalInput")
    ident_d = nc.dram_tensor("ident", [128, 128], F32, kind="ExternalInput")
    identb_d = nc.dram_tensor("identb", [128, 128], BF16, kind="ExternalInput")
    ones_d = nc.dram_tensor("ones1", [1, 128], F32, kind="ExternalInput")
    if has_pca_b:
        pca_b_d = nc.dram_tensor("pca_b", [1, 512], BF16, kind="ExternalInput")
        onesb_d = nc.dram_tensor("ones1b", [1, 128], BF16, kind="ExternalInput")
    else:
        onesb_d = None
    out_d = nc.dram_tensor("out", [TPC, 40], F32, kind="ExternalOutput")

    with TileProgram(nc, lay, has_pca_b) as tp:
        tp.run(xgt_d, pca_w_d, pprs_d, maskn_d, mlp_w_d, mlp_b_d, ident_d,
               identb_d, ones_d, pca_b_d if has_pca_b else None, onesb_d, out_d)
    nc.compile()
    return nc


class TileProgram:
    def __init__(self, nc, lay, has_pca_b):
        self.nc = nc
        self.lay = lay
        self.has_pca_b = has_pca_b
        self.ctx = ExitStack()
        self.tc_cm = tile.TileContext(nc)

    def __enter__(self):
        self.tc = self.tc_cm.__enter__()
        return self

    def __exit__(self, *exc):
        try:
            if exc[0] is None:
                self.ctx.close()
        finally:
            return self.tc_cm.__exit__(*exc)

    def pool(self, name, bufs, space="SBUF"):
        return self.ctx.enter_context(
            self.tc.tile_pool(name=name, bufs=bufs, space=space))

    def run(self, xgt_d, pca_w_d, pprs_d, maskn_d, mlp_w_d, mlp_b_d, ident_d,
            identb_d, ones_d, pca_b_d, onesb_d, out_d):
        nc, lay = self.nc, self.lay
        SJ, J, coff = lay.SJ, lay.J, lay.coff
        NS = lay.NSLOT

        consts = self.pool("consts", 1)
        resid = self.pool("resid", 1)
        xgp = self.pool("xgt", 2)
        trans = self.pool("trans", 1)
        small = self.pool("small", 2)
        slabs = self.pool("slabs", 1)
        psum_pca = self.pool("psum_pca", 2, space="PSUM")
        psum_u = self.pool("psum_u", 2, space="PSUM")
        psum_t = self.pool("psum_t", 2, space="PSUM")
        psum_l = self.pool("psum_l", 2, space="PSUM")

        # ---------------- constants / prologue loads ----------------
        pca_w_sb = []
        for kc in range(4):
            t = consts.tile([128, 512], BF16, tag=f"pcaw{kc}")
            nc.sync.dma_start(t[:], pca_w_d[kc * 128:(kc + 1) * 128, :])
            pca_w_sb.append(t)
        mlp_w_sb = []
        for kc in range(4):
            t = consts.tile([128, 40], F32, tag=f"mlpw{kc}")
            nc.sync.dma_start(t[:], mlp_w_d[kc * 128:(kc + 1) * 128, :])
            mlp_w_sb.append(t)
        mlp_b_sb = consts.tile([1, 40], F32, tag="mlpb")
        nc.sync.dma_start(mlp_b_sb[:], mlp_b_d[:, :])
        ident = consts.tile([128, 128], F32, tag="ident")
        nc.sync.dma_start(ident[:], ident_d[:, :])
        identb = consts.tile([128, 128], BF16, tag="identb")
        nc.sync.dma_start(identb[:], identb_d[:, :])
        ones1 = consts.tile([1, 128], F32, tag="ones1")
        nc.sync.dma_start(ones1[:], ones_d[:, :])
        if self.has_pca_b:
            pca_b_sb = consts.tile([1, 512], BF16, tag="pcab")
            nc.sync.dma_start(pca_b_sb[:], pca_b_d[:, :])
            onesb_sb = consts.tile([1, 128], BF16, tag="onesb")
            nc.sync.dma_start(onesb_sb[:], onesb_d[:, :])

        pprs = resid.tile([128, SJ], F32, tag="pprs")
        nc.sync.dma_start(pprs[:], pprs_d[:, :])
        maskn = resid.tile([128, SJ], F32, tag="maskn")
        nc.sync.dma_start(maskn[:], maskn_d[:, :])

        # Residents
        Zp = resid.tile([128, SJ * 512], BF16, tag="Zp")
        ubf = resid.tile([128, NTILES * 512], BF16, tag="ubf")
        rs = resid.tile([128, SJ * 8], F32, tag="rs")
        eppr = resid.tile([128, SJ], F32, tag="eppr")
        pprw = resid.tile([128, SJ], F32, tag="pprw")
        rS0 = resid.tile([128, NTILES], F32, tag="rS0")
        sig = resid.tile([128, NTILES * 8], F32, tag="sig")

        # Spread DMA-completion waits: touch DMA'd inputs once on DVE so later
        # consumers need no extra sync-wait slots (TT ISA allows few waits).
        touch = small.tile([128, 1], F32, tag="touch")
        nc.vector.tensor_copy(touch[:], maskn[:, 0:1])
        nc.vector.tensor_copy(touch[:], pprs[:, 0:1])

        # eppr = exp(pprs)  (pad slots hold -40 -> ~0)
        nc.scalar.activation(eppr[:], pprs[:], ACTF.Exp)
        for tl in range(NTILES):
            c0, c1 = int(coff[tl]), int(coff[tl + 1])
            s0 = small.tile([128, 1], F32, tag="s0")
            nc.vector.reduce_sum(s0[:], eppr[:, c0:c1], axis=AX.X)
            nc.vector.tensor_scalar_add(s0[:], s0[:], EPS)
            nc.vector.reciprocal(rS0[:, tl:tl + 1], s0[:])
            # pprw' = (1-beta) * eppr * recipS0 + maskneg
            tmp = small.tile([128, max(J)], F32, tag="pprwtmp")
            nc.vector.tensor_scalar(
                tmp[:, :c1 - c0], eppr[:, c0:c1], rS0[:, tl:tl + 1], 1.0 - BETA,
                op0=ALU.mult, op1=ALU.mult)
            nc.vector.tensor_add(pprw[:, c0:c1], tmp[:, :c1 - c0], maskn[:, c0:c1])

        # ---------------- PCA phase ----------------
        # For each slot-column: z = relu(xg @ pca_w [+ pca_b]); capsule sq-norms.
        ngroups = (SJ + 3) // 4
        for g in range(ngroups):
            w = min(4, SJ - g * 4)
            xts = []
            for kc in range(4):
                xt = xgp.tile([128, 512], BF16, tag=f"xgt{kc}")
                nc.sync.dma_start(
                    xt[:, :w * 128],
                    xgt_d[kc * 128:(kc + 1) * 128, g * 512:g * 512 + w * 128])
                xts.append(xt)
            for j4 in range(w):
                sc = g * 4 + j4
                ps = psum_pca.tile([128, 512], F32, tag="pca")
                nmm = 5 if self.has_pca_b else 4
                for kc in range(4):
                    nc.tensor.matmul(
                        ps[:], xts[kc][:, j4 * 128:(j4 + 1) * 128], pca_w_sb[kc][:],
                        start=(kc == 0), stop=(kc == nmm - 1))
                if self.has_pca_b:
                    nc.tensor.matmul(ps[:], onesb_sb[:], pca_b_sb[:],
                                     start=False, stop=True)
                zcol = Zp[:, sc * 512:(sc + 1) * 512]
                nc.scalar.activation(zcol, ps[:], ACTF.Relu)
                sq = slabs.tile([128, 512], BF16, tag="sq")
                nc.vector.tensor_mul(sq[:], zcol, zcol)
                ssq = rs[:, sc * 8:(sc + 1) * 8]
                nc.vector.reduce_sum(
                    ssq, sq[:].rearrange("p (k d) -> p k d", d=64), axis=AX.X)
        # rs = 1 / max(sqrt(ssq), 1e-12)   (batched in place over the slab)
        nc.scalar.activation(rs[:], rs[:], ACTF.Sqrt)
        nc.vector.tensor_scalar_max(rs[:], rs[:], 1e-12)
        nc.vector.reciprocal(rs[:], rs[:])

        # ---------------- routing ----------------
        # it == -1 is the ppr-weighted init scatter; 0..2 are routing rounds.
        for it in range(-1, ROUIT):
            for tl in range(NTILES):
                self.tile_round(it, tl, Zp, ubf, rs, eppr, pprw, maskn, rS0, sig,
                                identb, small, slabs, psum_u, psum_t, psum_l,
                                mlp_w_sb, mlp_b_sb, ones1, ident, out_d)

    def tile_round(self, it, tl, Zp, ubf, rs, eppr, pprw, maskn, rS0, sig,
                   identb, small, slabs, psum_u, psum_t, psum_l, mlp_w_sb, mlp_b_sb,
                   ones1, ident, out_d):
        nc, lay = self.nc, self.lay
        J = lay.J[tl]
        c0 = int(lay.coff[tl])
        Jm = max(lay.J)
        last = (it == ROUIT - 1)

        ups = psum_u.tile([128, 512], F32, tag="upsum")

        if it < 0:
            # init: weights w0 = eppr (recipS0 folded into sigma_init later)
            e2p = small.tile([128, Jm * 8], F32, tag="e2p")
            nc.vector.tensor_mul(
                e2p[:, :J * 8].rearrange("p (j k) -> p j k", k=8),
                rs[:, c0 * 8:(c0 + J) * 8].rearrange("p (j k) -> p j k", k=8),
                eppr[:, c0:c0 + J].unsqueeze(2).broadcast_to((128, J, 8)))
            self.weighted_scatter(tl, Zp, e2p, identb, slabs, ups, J, c0)
        else:
            # sigma for this round: recipS0 (it=0) or 1/||u_prev|| (it>0)
            e1 = small.tile([128, Jm * 8], F32, tag="e1")
            rssig = small.tile([128, Jm * 8], F32, tag="rssig")
            if it == 0:
                sig_ap = rS0[:, tl:tl + 1].unsqueeze(2).broadcast_to((128, J, 8))
            else:
                sig_ap = sig[:, tl * 8:(tl + 1) * 8].unsqueeze(1).broadcast_to(
                    (128, J, 8))
            nc.vector.tensor_mul(
                rssig[:, :J * 8].rearrange("p (j k) -> p j k", k=8),
                rs[:, c0 * 8:(c0 + J) * 8].rearrange("p (j k) -> p j k", k=8),
                sig_ap)
            ub = ubf[:, tl * 512:(tl + 1) * 512]
            for (j0, jc) in chunks_of(J):
                m1 = slabs.tile([128, JC * 512], BF16, tag="m1")
                nc.vector.tensor_mul(
                    m1[:, :jc * 512].rearrange("p (j f) -> p j f", f=512),
                    Zp[:, (c0 + j0) * 512:(c0 + j0 + jc) * 512].rearrange(
                        "p (j f) -> p j f", f=512),
                    ub.unsqueeze(1).broadcast_to((128, jc, 512)))
                praw = small.tile([128, JC * 8], F32, tag="praw")
                nc.vector.reduce_sum(
                    praw[:, :jc * 8],
                    m1[:, :jc * 512].rearrange("p (j k d) -> p j k d", k=8, d=64),
                    axis=AX.X)
                nc.vector.tensor_mul(praw[:, :jc * 8], praw[:, :jc * 8],
                                     rssig[:, j0 * 8:(j0 + jc) * 8])
                nc.vector.tensor_add(
                    praw[:, :jc * 8].rearrange("p (j k) -> p j k", k=8),
                    praw[:, :jc * 8].rearrange("p (j k) -> p j k", k=8),
                    maskn[:, c0 + j0:c0 + j0 + jc].unsqueeze(2).broadcast_to(
                        (128, jc, 8)))
                nc.scalar.activation(e1[:, j0 * 8:(j0 + jc) * 8],
                                     praw[:, :jc * 8], ACTF.Exp)
            # S1 softmax denominator; r1b = beta / (S1 + eps)
            s1 = small.tile([128, 8], F32, tag="s1")
            nc.vector.reduce_sum(
                s1[:], e1[:, :J * 8].rearrange("p (j k) -> p k j", k=8), axis=AX.X)
            nc.vector.tensor_scalar_add(s1[:], s1[:], EPS)
            r1b = small.tile([128, 8], F32, tag="r1b")
            nc.vector.reciprocal(r1b[:], s1[:])
            nc.vector.tensor_scalar_mul(r1b[:], r1b[:], BETA)

            e2 = small.tile([128, Jm * 8], F32, tag="e2")
            e2p = small.tile([128, Jm * 8], F32, tag="e2p")
            for (j0, jc) in chunks_of(J):
                p2 = small.tile([128, JC * 8], F32, tag="p2")
                nc.vector.tensor_mul(
                    p2[:, :jc * 8].rearrange("p (j k) -> p j k", k=8),
                    e1[:, j0 * 8:(j0 + jc) * 8].rearrange("p (j k) -> p j k", k=8),
                    r1b[:].unsqueeze(1).broadcast_to((128, jc, 8)))
                nc.vector.tensor_add(
                    p2[:, :jc * 8].rearrange("p (j k) -> p j k", k=8),
                    p2[:, :jc * 8].rearrange("p (j k) -> p j k", k=8),
                    pprw[:, c0 + j0:c0 + j0 + jc].unsqueeze(2).broadcast_to(
                        (128, jc, 8)))
                nc.scalar.activation(e2[:, j0 * 8:(j0 + jc) * 8],
                                     p2[:, :jc * 8], ACTF.Exp)
                nc.vector.tensor_mul(e2p[:, j0 * 8:(j0 + jc) * 8],
                                     e2[:, j0 * 8:(j0 + jc) * 8],
                                     rs[:, (c0 + j0) * 8:(c0 + j0 + jc) * 8])
            self.weighted_scatter(tl, Zp, e2p, identb, slabs, ups, J, c0)

        if it < ROUIT - 1:
            # next u in bf16 + sigma = 1/max(||u_k||, 1e-12) from psum
            sq = slabs.tile([128, 512], BF16, tag="sq")
            nc.scalar.activation(sq[:], ups[:], ACTF.Square)
            ss = small.tile([128, 8], F32, tag="uss")
            nc.vector.reduce_sum(ss[:], sq[:].rearrange("p (k d) -> p k d", d=64),
                                 axis=AX.X)
            if it >= 0:
                sg = sig[:, tl * 8:(tl + 1) * 8]
                nc.scalar.activation(sg, ss[:], ACTF.Sqrt)
                nc.vector.tensor_scalar_max(sg, sg, 1e-12)
                nc.vector.reciprocal(sg, sg)
            nc.scalar.activation(ubf[:, tl * 512:(tl + 1) * 512], ups[:], ACTF.Copy)
        else:
            # final: u = relu(u_raw) * recipS2 per capsule; logits; log_softmax
            e2s = small.tile([128, 8], F32, tag="s2")
            nc.vector.reduce_sum(
                e2s[:], e2[:, :J * 8].rearrange("p (j k) -> p k j", k=8), axis=AX.X)
            nc.vector.tensor_scalar_add(e2s[:], e2s[:], EPS)
            rS2 = small.tile([128, 8], F32, tag="rs2")
            nc.vector.reciprocal(rS2[:], e2s[:])
            usc = slabs.tile([128, 512], F32, tag="usc")
            for k in range(8):
                nc.scalar.activation(usc[:, k * 64:(k + 1) * 64],
                                     ups[:, k * 64:(k + 1) * 64], ACTF.Relu,
                                     scale=rS2[:, k:k + 1])
            lg = psum_l.tile([128, 40], F32, tag="logits")
            uts = []
            for ch in range(4):
                tp = psum_t.tile([128, 128], F32, tag="tpos")
                nc.tensor.transpose(tp[:], usc[:, ch * 128:(ch + 1) * 128], ident[:])
                ut = slabs.tile([128, 128], F32, tag=f"ut{ch}")
                nc.scalar.activation(ut[:], tp[:], ACTF.Copy)
                uts.append(ut)
            for ch in range(4):
                nc.tensor.matmul(lg[:], uts[ch][:], mlp_w_sb[ch][:],
                                 start=(ch == 0), stop=False)
            nc.tensor.matmul(lg[:], ones1[:], mlp_b_sb[:], start=False, stop=True)
            mx = small.tile([128, 1], F32, tag="mx")
            nc.vector.reduce_max(mx[:], lg[:], axis=AX.X)
            nc.vector.tensor_scalar_mul(mx[:], mx[:], -1.0)
            ex = small.tile([128, 40], F32, tag="ex")
            se = small.tile([128, 1], F32, tag="se")
            nc.scalar.activation(ex[:], lg[:], ACTF.Exp, bias=mx[:, 0:1],
                                 accum_out=se[:])
            lse = small.tile([128, 1], F32, tag="lse")
            nc.scalar.activation(lse[:], se[:], ACTF.Ln)
            ob = small.tile([128, 40], F32, tag="ob")
            nc.vector.tensor_scalar(ob[:], lg[:], mx[:, 0:1], lse[:, 0:1],
                                    op0=ALU.add, op1=ALU.subtract)
            nc.sync.dma_start(out_d[tl * 128:(tl + 1) * 128, :], ob[:])

    def weighted_scatter(self, tl, Zp, e2p, identb, slabs, ups, J, c0):
        """u_psum[t, :] = sum_j e2p[t, j, k] * Zp[t, j, :] via PE identity-matmul."""
        nc = self.nc
        for (j0, jc) in chunks_of(J):
            w = slabs.tile([128, JC * 512], BF16, tag="w")
            nc.vector.tensor_mul(
                w[:, :jc * 512].rearrange("p (j k d) -> p j k d", k=8, d=64),
                Zp[:, (c0 + j0) * 512:(c0 + j0 + jc) * 512].rearrange(
                    "p (j k d) -> p j k d", k=8, d=64),
                e2p[:, j0 * 8:(j0 + jc) * 8].rearrange(
                    "p (j k) -> p j k", k=8).unsqueeze(3).broadcast_to(
                        (128, jc, 8, 64)))
            for j in range(jc):
                nc.tensor.matmul(ups[:], identb[:], w[:, j * 512:(j + 1) * 512],
                                 start=(j0 + j == 0), stop=(j0 + j == J - 1))


# ----------------------------------------------------------------------------
# Entry point
# ----------------------------------------------------------------------------

_CACHE = {}


def _prepare(x_nb, ppr, pca_w, pca_b, mlp_w, mlp_b, row_idx, col_idx, x_idx):
    lay = build_layout(row_idx, col_idx, ppr)
    has_pca_b = bool(np.any(pca_b))
    nc = build_program(lay, has_pca_b)
    in_maps = []
    shared = {
        "pca_w": np.ascontiguousarray(pca_w).astype(bf16),
        "mlp_w": np.ascontiguousarray(mlp_w).astype(np.float32),
        "mlp_b": np.ascontiguousarray(mlp_b).reshape(1, 40).astype(np.float32),
        "ident": np.eye(128, dtype=np.float32),
        "identb": np.eye(128).astype(bf16),
        "ones1": np.ones((1, 128), dtype=np.float32),
    }
    if has_pca_b:
        shared["pca_b"] = np.ascontiguousarray(pca_b).reshape(1, 512).astype(bf16)
        shared["ones1b"] = np.ones((1, 128), dtype=bf16)
    for c in range(NCORES):
        m = dict(shared)
        m.update(build_core_inputs(lay, c, x_nb, col_idx, ppr))
        in_maps.append(m)
    return lay, nc, in_maps


def _assemble(lay, results):
    out = np.empty((T, 40), dtype=np.float32)
    for c in range(NCORES):
        order = lay.cores[c]["order"]
        out[c * TPC + order] = results[c]["out"]
    return out


def kernel(**inputs):
    inputs = {k: np.asarray(v) for k, v in inputs.items()}
    lay, nc, in_maps = _prepare(**inputs)
    res = run_bass_kernel_spmd(nc, in_maps, list(range(NCORES)))
    return _assemble(lay, res.results)


# -- timing helper for test.py (not used by the grading harness) --------------

def bench(iters=10, **inputs):
    """Returns (output, best_ns) using a persistent jitted executable."""
    import jax
    from jax.sharding import Mesh, PartitionSpec
    from jax.experimental.shard_map import shard_map
    from concourse import bass2jax

    inputs = {k: np.asarray(v) for k, v in inputs.items()}
    lay, nc, in_maps = _prepare(**inputs)

    bass2jax.install_neuronx_cc_hook()
    partition_name = (nc.partition_id_tensor.name
                      if nc.partition_id_tensor else None)
    in_names, out_names, out_avals, zero_outs = [], [], [], []
    for alloc in nc.m.functions[0].allocations:
        if not isinstance(alloc, mybir.MemoryLocationSet):
            continue
        name = alloc.memorylocations[0].name
        if alloc.kind == "ExternalInput":
            if name != partition_name:
                in_names.append(name)
        elif alloc.kind == "ExternalOutput":
            out_names.append(name)
            shape = tuple(alloc.tensor_shape)
            dtype = mybir.dt.np(alloc.dtype)
            out_avals.append(jax.core.ShapedArray(shape, dtype))
            zero_outs.append(np.zeros(shape, dtype))
    n_params = len(in_names)
    n_outs = len(out_avals)
    all_names = list(in_names) + list(out_names)
    if partition_name is not None:
        all_names.append(partition_name)

    def _body(*args):
        operands = list(args)
        if partition_name is not None:
            operands.append(bass2jax.partition_id_tensor())
        outs = bass2jax._bass_exec_p.bind(
            *operands, out_avals=tuple(out_avals), in_names=tuple(all_names),
            out_names=tuple(out_names), lowering_input_output_aliases=(),
            sim_require_finite=True, sim_require_nnan=True, nc=nc)
        return tuple(outs)

    devices = jax.devices()[:NCORES]
    mesh = Mesh(np.asarray(devices), ("core",))
    donate = tuple(range(n_params, n_params + n_outs))
    sharded = jax.jit(
        shard_map(_body, mesh=mesh,
                  in_specs=(PartitionSpec("core"),) * (n_params + n_outs),
                  out_specs=(PartitionSpec("core"),) * n_outs,
                  check_rep=False),
        donate_argnums=donate, keep_unused=True)

    concat_in = [
        np.concatenate([np.asarray(in_maps[c][nm]) for c in range(NCORES)], axis=0)
        for nm in in_names]
    dev_in = [jax.device_put(a) for a in concat_in]

    def zeros():
        return [jax.device_put(np.zeros((NCORES * z.shape[0], *z.shape[1:]),
                                        z.dtype)) for z in zero_outs]

    out_arrs = sharded(*dev_in, *zeros())          # warmup + correctness
    jax.block_until_ready(out_arrs)
    results = [
        {nm: np.asarray(out_arrs[i]).reshape(NCORES, *out_avals[i].shape)[c]
         for i, nm in enumerate(out_names)}
        for c in range(NCORES)]
    output = _assemble(lay, results)

    best = float("inf")
    for _ in range(iters):
        zs = zeros()
        jax.block_until_ready(zs)
        t0 = time.perf_counter()
        o = sharded(*dev_in, *zs)
        jax.block_until_ready(o)
        best = min(best, time.perf_counter() - t0)
    return output, int(best * 1e9)


if __name__ == "__main__":
    import reference
    ins = {k: np.asarray(v) for k, v in reference.setup_inputs().items()}
    out = kernel(**ins)
    exp = np.asarray(reference.reference(**ins))
    err = np.abs(out - exp).max()
    print("max abs err:", err, "absmax:", np.abs(exp).max())


# revision 37
# speedup vs baseline: 114.7669x; 114.7669x over previous
"""Trainium2 Bass kernel for capsule-routing GNN message passing.

Problem: nn_COSAL_33981781246135 (gnn_message_passing).

Strategy (graph/data parallel, per the sharding hint):
  - Targets are sharded contiguously across the 8 cores (2048 targets each).
  - Each core receives its incident edges' neighbor rows pre-gathered on the
    host (x_nb[col_idx] for its edge range), already transposed + cast to bf16.
  - On-device layout is target-major "slot" form: each 128-target tile has its
    targets on partitions and its edges padded to J slots along the free dim.
    Targets are degree-sorted on the host so J is near the tile's mean degree.
    All segment ops (softmax sums, scatter-adds) become free-dim reduces or
    PE identity-matmul PSUM accumulations - no one-hot matmuls, no gathers.
  - All per-(target,capsule) normalizations (1/S softmax denominators, capsule
    l2 norms) are algebraically folded into the next per-edge logit scale, so
    z and u are kept raw in bf16 and never rescaled in memory.
"""

import os
import sys
import time

for _p in ("/opt/trn_rl_repo", os.path.expanduser("~/.axon_site/_ro/trn_rl_repo")):
    if os.path.isdir(_p) and _p not in sys.path:
        sys.path.insert(0, _p)

import numpy as np
import ml_dtypes
from contextlib import ExitStack

import concourse.bass as bass
import concourse.bacc as bacc
import concourse.mybir as mybir
from concourse import tile
from concourse.bass_utils import run_bass_kernel_spmd

BF16 = mybir.dt.bfloat16
F32 = mybir.dt.float32
AX = mybir.AxisListType
ALU = mybir.AluOpType
ACTF = mybir.ActivationFunctionType

NCORES = 8
K = 8          # capsules
DD = 64        # per-capsule dim
D = 512
T = 16384      # targets
NB = 100000
E = 131072
TPC = T // NCORES        # 2048 targets per core
NTILES = TPC // 128      # 16 tiles per core
ROUIT = 3
BETA = 0.5
JC = 11                  # slot-columns per chunk in the routing loop
MASKNEG = -40.0
EPS = 1e-6

bf16 = ml_dtypes.bfloat16


# ----------------------------------------------------------------------------
# Host-side layout construction
# ----------------------------------------------------------------------------

class Layout:
    pass


def build_layout(row_idx, col_idx, ppr):
    """Compute the unified slot layout + per-core input tensors."""
    lay = Layout()
    bounds = np.searchsorted(row_idx, np.arange(NCORES + 1) * TPC).astype(np.int64)
    cores = []
    for c in range(NCORES):
        e0, e1 = int(bounds[c]), int(bounds[c + 1])
        r = row_idx[e0:e1].astype(np.int64) - c * TPC
        deg = np.bincount(r, minlength=TPC)
        order = np.argsort(-deg, kind="stable")
        inv_order = np.empty(TPC, dtype=np.int64)
        inv_order[order] = np.arange(TPC)
        cores.append((e0, e1, r, deg, order, inv_order))

    # Unified per-tile slot count J (max over cores so one program fits all).
    J = []
    for t in range(NTILES):
        m = 1
        for (_, _, _, deg, order, _) in cores:
            m = max(m, int(deg[order[t * 128:(t + 1) * 128]].max()))
        J.append(m)
    lay.J = J
    lay.SJ = int(sum(J))
    lay.NSLOT = 128 * lay.SJ
    lay.coff = np.concatenate([[0], np.cumsum(J)]).astype(np.int64)  # col offsets

    # Map each slot-column to (tile, j) for the builder.
    col2tile = []
    for t in range(NTILES):
        for j in range(J[t]):
            col2tile.append((t, j))
    lay.col2tile = col2tile

    lay.cores = []
    for (e0, e1, r, deg, order, inv_order) in cores:
        ec = e1 - e0
        # Edge -> slot position. Edges are sorted by r, so the rank of an edge
        # within its target is e - start[r[e]].
        starts = np.concatenate([[0], np.cumsum(deg)]).astype(np.int64)
        eloc = np.arange(ec, dtype=np.int64)
        jrank = eloc - starts[r]
        pos = inv_order[r]                       # position in degree-sorted order
        tl = pos // 128
        part = pos % 128
        col = lay.coff[tl] + jrank               # global slot-column
        slot = col * 128 + part                  # flat slot id
        eid = np.full(lay.NSLOT, -1, dtype=np.int64)
        eid[slot] = eloc
        cd = {}
        cd["e0"], cd["e1"] = e0, e1
        cd["order"] = order
        cd["eid"] = eid
        lay.cores.append(cd)
    return lay


def build_core_inputs(lay, c, x_nb, col_idx, ppr):
    cd = lay.cores[c]
    e0, eid = cd["e0"], cd["eid"]
    valid = eid >= 0
    cols = np.where(valid, col_idx[e0:][np.maximum(eid, 0)], 0)
    xg = x_nb[cols]                              # (NSLOT, 512) f32
    xgt = np.ascontiguousarray(xg.T).astype(bf16)  # (512, NSLOT)
    pprs = np.where(valid, ppr[e0:][np.maximum(eid, 0)], MASKNEG).astype(np.float32)
    pprs = np.ascontiguousarray(pprs.reshape(lay.SJ, 128).T)      # (128, SJ)
    maskn = np.where(valid, 0.0, MASKNEG).astype(np.float32)
    maskn = np.ascontiguousarray(maskn.reshape(lay.SJ, 128).T)    # (128, SJ)
    return {"xgt": xgt, "pprs": pprs, "maskn": maskn}


# ----------------------------------------------------------------------------
# Device program
# ----------------------------------------------------------------------------

def chunks_of(J):
    out = []
    c0 = 0
    while c0 < J:
        out.append((c0, min(JC, J - c0)))
        c0 += JC
    return out


def build_program(lay, has_pca_b):
    """Build with the configured chunk width, backing off if SBUF overflows
    (larger-than-expected slot counts on unusual degree distributions)."""
    global JC
    last = None
    for jc_try in (11, 9, 7, 5, 3, 2, 1):
        JC = jc_try
        try:
            return _build_program(lay, has_pca_b)
        except ValueError as e:
            if "Not enough space" not in str(e):
                raise
            last = e
    raise last


def _build_program(lay, has_pca_b):
    nc = bacc.Bacc("TRN2", target_bir_lowering=False, debug=False)
    SJ, J, coff = lay.SJ, lay.J, lay.coff

    # DRAM I/O
    xgt_d = nc.dram_tensor("xgt", [512, lay.NSLOT], BF16, kind="ExternalInput")
    pca_w_d = nc.dram_tensor("pca_w", [512, 512], BF16, kind="ExternalInput")
    pprs_d = nc.dram_tensor("pprs", [128, SJ], F32, kind="ExternalInput")
    maskn_d = nc.dram_tensor("maskn", [128, SJ], F32, kind="ExternalInput")
    mlp_w_d = nc.dram_tensor("mlp_w", [512, 40], F32, kind="ExternalInput")
    mlp_b_d = nc.dram_tensor("mlp_b", [1, 40], F32, kind="ExternalInput")
    ident_d = nc.dram_tensor("ident", [128, 128], F32, kind="ExternalInput")
    identb_d = nc.dram_tensor("identb", [128, 128], BF16, kind="ExternalInput")
    ones_d = nc.dram_tensor("ones1", [1, 128], F32, kind="ExternalInput")
    if has_pca_b:
        pca_b_d = nc.dram_tensor("pca_b", [1, 512], BF16, kind="ExternalInput")
        onesb_d = nc.dram_tensor("ones1b", [1, 128], BF16, kind="ExternalInput")
    else:
        onesb_d = None
    out_d = nc.dram_tensor("out", [TPC, 40], F32, kind="ExternalOutput")

    with TileProgram(nc, lay, has_pca_b) as tp:
        tp.run(xgt_d, pca_w_d, pprs_d, maskn_d, mlp_w_d, mlp_b_d, ident_d,
               identb_d, ones_d, pca_b_d if has_pca_b else None, onesb_d, out_d)
    nc.compile()
    return nc


class TileProgram:
    def __init__(self, nc, lay, has_pca_b):
        self.nc = nc
        self.lay = lay
        self.has_pca_b = has_pca_b
        self.ctx = ExitStack()
        self.tc_cm = tile.TileContext(nc)

    def __enter__(self):
        self.tc = self.tc_cm.__enter__()
        return self

    def __exit__(self, *exc):
        try:
            if exc[0] is None:
                self.ctx.close()
        finally:
            return self.tc_cm.__exit__(*exc)

    def pool(self, name, bufs, space="SBUF"):
        return self.ctx.enter_context(
            self.tc.tile_pool(name=name, bufs=bufs, space=space))

    def run(self, xgt_d, pca_w_d, pprs_d, maskn_d, mlp_w_d, mlp_b_d, ident_d,
            identb_d, ones_d, pca_b_d, onesb_d, out_d):
        nc, lay = self.nc, self.lay
        SJ, J, coff = lay.SJ, lay.J, lay.coff
        NS = lay.NSLOT

        consts = self.pool("consts", 1)
        resid = self.pool("resid", 1)
        xgp = self.pool("xgt", 2)
        trans = self.pool("trans", 1)
        small = self.pool("small", 2)
        slabs = self.pool("slabs", 1)
        psum_pca = self.pool("psum_pca", 2, space="PSUM")
        psum_u = self.pool("psum_u", 2, space="PSUM")
        psum_t = self.pool("psum_t", 2, space="PSUM")
        psum_l = self.pool("psum_l", 2, space="PSUM")

        # ---------------- constants / prologue loads ----------------
        pca_w_sb = []
        for kc in range(4):
            t = consts.tile([128, 512], BF16, tag=f"pcaw{kc}")
            nc.sync.dma_start(t[:], pca_w_d[kc * 128:(kc + 1) * 128, :])
            pca_w_sb.append(t)
        mlp_w_sb = []
        for kc in range(4):
            t = consts.tile([128, 40], F32, tag=f"mlpw{kc}")
            nc.sync.dma_start(t[:], mlp_w_d[kc * 128:(kc + 1) * 128, :])
            mlp_w_sb.append(t)
        mlp_b_sb = consts.tile([1, 40], F32, tag="mlpb")
        nc.sync.dma_start(mlp_b_sb[:], mlp_b_d[:, :])
        ident = consts.tile([128, 128], F32, tag="ident")
        nc.sync.dma_start(ident[:], ident_d[:, :])
        identb = consts.tile([128, 128], BF16, tag="identb")
        nc.sync.dma_start(identb[:], identb_d[:, :])
        ones1 = consts.tile([1, 128], F32, tag="ones1")
        nc.sync.dma_start(ones1[:], ones_d[:, :])
        if self.has_pca_b:
            pca_b_sb = consts.tile([1, 512], BF16, tag="pcab")
            nc.sync.dma_start(pca_b_sb[:], pca_b_d[:, :])
            onesb_sb = consts.tile([1, 128], BF16, tag="onesb")
            nc.sync.dma_start(onesb_sb[:], onesb_d[:, :])

        pprs = resid.tile([128, SJ], F32, tag="pprs")
        nc.sync.dma_start(pprs[:], pprs_d[:, :])
        maskn = resid.tile([128, SJ], F32, tag="maskn")
        nc.sync.dma_start(maskn[:], maskn_d[:, :])

        dramp = self.pool("dram", 1, space="DRAM")
        ubf = [dramp.tile([128, 512], BF16, tag=f"ustate{tl}",
                          name=f"ustate{tl}") for tl in range(NTILES)]
        rs = [resid.tile([128, J[tl] * 8], F32, tag=f"rs{tl}",
                         name=f"rs{tl}") for tl in range(NTILES)]
        eppr = resid.tile([128, SJ], F32, tag="eppr")
        pprw = resid.tile([128, SJ], F32, tag="pprw")
        rS0 = resid.tile([128, NTILES], F32, tag="rS0")
        sig = [resid.tile([128, 8], F32, tag=f"sig{tl}", name=f"sig{tl}")
               for tl in range(NTILES)]

        # Spread DMA-completion waits: touch DMA'd inputs once on DVE so later
        # consumers need no extra sync-wait slots (TT ISA allows few waits).
        touch = small.tile([128, 1], F32, tag="touch")
        nc.vector.tensor_copy(touch[:], maskn[:, 0:1])
        nc.vector.tensor_copy(touch[:], pprs[:, 0:1])

        # eppr = exp(pprs)  (pad slots hold -40 -> ~0)
        nc.scalar.activation(eppr[:], pprs[:], ACTF.Exp)
        for tl in range(NTILES):
            c0, c1 = int(coff[tl]), int(coff[tl + 1])
            s0 = small.tile([128, 1], F32, tag="s0")
            nc.vector.reduce_sum(s0[:], eppr[:, c0:c1], axis=AX.X)
            nc.vector.tensor_scalar_add(s0[:], s0[:], EPS)
            nc.vector.reciprocal(rS0[:, tl:tl + 1], s0[:])
            # pprw' = (1-beta) * eppr * recipS0 + maskneg
            tmp = small.tile([128, max(J)], F32, tag="pprwtmp")
            nc.vector.tensor_scalar(
                tmp[:, :c1 - c0], eppr[:, c0:c1], rS0[:, tl:tl + 1], 1.0 - BETA,
                op0=ALU.mult, op1=ALU.mult)
            nc.vector.tensor_add(pprw[:, c0:c1], tmp[:, :c1 - c0], maskn[:, c0:c1])

        Zp = [resid.tile([128, J[tl] * 512], BF16, tag=f"Zp{tl}",
                         name=f"Zp{tl}") for tl in range(NTILES)]

        # ------- PCA per tile, with the init scatter hoisted in (fills DVE) ---
        for tl in range(NTILES):
            Jt, c0 = J[tl], int(coff[tl])
            zp = Zp[tl]
            for g0 in range(0, Jt, 4):
                w = min(4, Jt - g0)
                xts = []
                for kc in range(4):
                    xt = xgp.tile([128, 512], BF16, tag=f"xgt{kc}")
                    nc.sync.dma_start(
                        xt[:, :w * 128],
                        xgt_d[kc * 128:(kc + 1) * 128,
                              (c0 + g0) * 128:(c0 + g0 + w) * 128])
                    xts.append(xt)
                for j4 in range(w):
                    ps = psum_pca.tile([128, 512], F32, tag="pca")
                    nmm = 5 if self.has_pca_b else 4
                    for kc in range(4):
                        nc.tensor.matmul(
                            ps[:], xts[kc][:, j4 * 128:(j4 + 1) * 128],
                            pca_w_sb[kc][:],
                            start=(kc == 0), stop=(kc == nmm - 1))
                    if self.has_pca_b:
                        nc.tensor.matmul(ps[:], onesb_sb[:], pca_b_sb[:],
                                         start=False, stop=True)
                    zcol = zp[:, (g0 + j4) * 512:(g0 + j4 + 1) * 512]
                    nc.scalar.activation(zcol, ps[:], ACTF.Relu)
                sqg = slabs.tile([128, 4 * 512], BF16, tag="sqg")
                nc.vector.tensor_mul(sqg[:, :w * 512],
                                     zp[:, g0 * 512:(g0 + w) * 512],
                                     zp[:, g0 * 512:(g0 + w) * 512])
                nc.vector.reduce_sum(
                    rs[tl][:, g0 * 8:(g0 + w) * 8],
                    sqg[:, :w * 512].rearrange("p (c d k) -> p c k d", d=64, k=8),
                    axis=AX.X)
            # rs = 1 / max(sqrt(ssq), 1e-12) for this tile
            nc.scalar.activation(rs[tl][:], rs[tl][:], ACTF.Sqrt)
            nc.vector.tensor_scalar_max(rs[tl][:], rs[tl][:], 1e-12)
            nc.vector.reciprocal(rs[tl][:], rs[tl][:])

        # ---------------- routing rounds ----------------
        for it in range(-1, ROUIT):
            for tl in range(NTILES):
                self.tile_round(it, tl, Zp[tl], ubf, rs, eppr, pprw, maskn, rS0,
                                sig, identb, small, slabs, psum_u, psum_t, psum_l,
                                mlp_w_sb, mlp_b_sb, ones1, ident, out_d)

    def tile_round(self, it, tl, zp, ubf, rs, eppr, pprw, maskn, rS0, sig,
                   identb, small, slabs, psum_u, psum_t, psum_l, mlp_w_sb, mlp_b_sb,
                   ones1, ident, out_d):
        nc, lay = self.nc, self.lay
        J = lay.J[tl]
        c0 = int(lay.coff[tl])
        Jm = max(lay.J)
        last = (it == ROUIT - 1)

        ups = psum_u.tile([128, 512], F32, tag="upsum")

        if it < 0:
            # init: weights w0 = eppr (recipS0 folded into sigma_init later)
            e2p = small.tile([128, Jm * 8], BF16, tag="e2p")
            nc.vector.tensor_mul(
                e2p[:, :J * 8].rearrange("p (j k) -> p j k", k=8),
                rs[tl][:].rearrange("p (j k) -> p j k", k=8),
                eppr[:, c0:c0 + J].unsqueeze(2).broadcast_to((128, J, 8)))
            self.weighted_scatter(tl, zp, e2p, identb, slabs, ups, J, c0)
        else:
            # sigma for this round: recipS0 (it=0) or 1/||u_prev|| (it>0)
            e1 = small.tile([128, Jm * 8], F32, tag="e1")
            rssig = small.tile([128, Jm * 8], F32, tag="rssig")
            if it == 0:
                sig_ap = rS0[:, tl:tl + 1].unsqueeze(2).broadcast_to((128, J, 8))
            else:
                sig_ap = sig[tl][:].unsqueeze(1).broadcast_to((128, J, 8))
            nc.vector.tensor_mul(
                rssig[:, :J * 8].rearrange("p (j k) -> p j k", k=8),
                rs[tl][:].rearrange("p (j k) -> p j k", k=8),
                sig_ap)
            ub = small.tile([128, 512], BF16, tag="ubr")
            nc.sync.dma_start(ub[:], ubf[tl][:, :])
            praw = small.tile([128, Jm * 8], F32, tag="praw")
            for (j0, jc) in chunks_of(J):
                m1 = slabs.tile([128, JC * 512], BF16, tag="m1")
                nc.vector.tensor_mul(
                    m1[:, :jc * 512].rearrange("p (j f) -> p j f", f=512),
                    zp[:, j0 * 512:(j0 + jc) * 512].rearrange(
                        "p (j f) -> p j f", f=512),
                    ub[:].unsqueeze(1).broadcast_to((128, jc, 512)))
                nc.vector.reduce_sum(
                    praw[:, j0 * 8:(j0 + jc) * 8],
                    m1[:, :jc * 512].rearrange("p (j d k) -> p j k d", k=8, d=64),
                    axis=AX.X)
            nc.vector.tensor_mul(praw[:, :J * 8], praw[:, :J * 8],
                                 rssig[:, :J * 8])
            nc.vector.tensor_add(
                praw[:, :J * 8].rearrange("p (j k) -> p j k", k=8),
                praw[:, :J * 8].rearrange("p (j k) -> p j k", k=8),
                maskn[:, c0:c0 + J].unsqueeze(2).broadcast_to((128, J, 8)))
            nc.scalar.activation(e1[:, :J * 8], praw[:, :J * 8], ACTF.Exp)
            # S1 softmax denominator; r1b = beta / (S1 + eps)
            s1 = small.tile([128, 8], F32, tag="s1")
            nc.vector.reduce_sum(
                s1[:], e1[:, :J * 8].rearrange("p (j k) -> p k j", k=8), axis=AX.X)
            nc.vector.tensor_scalar_add(s1[:], s1[:], EPS)
            r1b = small.tile([128, 8], F32, tag="r1b")
            nc.vector.reciprocal(r1b[:], s1[:])
            nc.vector.tensor_scalar_mul(r1b[:], r1b[:], BETA)

            e2 = small.tile([128, Jm * 8], F32, tag="e2")
            e2p = small.tile([128, Jm * 8], BF16, tag="e2p")
            p2 = small.tile([128, Jm * 8], F32, tag="p2")
            nc.vector.tensor_mul(
                p2[:, :J * 8].rearrange("p (j k) -> p j k", k=8),
                e1[:, :J * 8].rearrange("p (j k) -> p j k", k=8),
                r1b[:].unsqueeze(1).broadcast_to((128, J, 8)))
            nc.vector.tensor_add(
                p2[:, :J * 8].rearrange("p (j k) -> p j k", k=8),
                p2[:, :J * 8].rearrange("p (j k) -> p j k", k=8),
                pprw[:, c0:c0 + J].unsqueeze(2).broadcast_to((128, J, 8)))
            nc.scalar.activation(e2[:, :J * 8], p2[:, :J * 8], ACTF.Exp)
            nc.vector.tensor_mul(e2p[:, :J * 8], e2[:, :J * 8],
                                 rs[tl][:])
            self.weighted_scatter(tl, zp, e2p, identb, slabs, ups, J, c0)

        if it < ROUIT - 1:
            # next u in bf16 + sigma = 1/max(||u_k||, 1e-12) from psum
            sq = slabs.tile([128, 512], BF16, tag="sqg")
            nc.scalar.activation(sq[:], ups[:], ACTF.Square)
            ss = small.tile([128, 8], F32, tag="uss")
            nc.vector.reduce_sum(ss[:], sq[:].rearrange("p (d k) -> p k d", k=8),
                                 axis=AX.X)
            if it >= 0:
                sg = sig[tl][:]
                nc.scalar.activation(sg, ss[:], ACTF.Sqrt)
                nc.vector.tensor_scalar_max(sg, sg, 1e-12)
                nc.vector.reciprocal(sg, sg)
            ubw = small.tile([128, 512], BF16, tag="ubw")
            nc.scalar.activation(ubw[:], ups[:], ACTF.Copy)
            nc.sync.dma_start(ubf[tl][:, :], ubw[:])
        else:
            # final: u = relu(u_raw) * recipS2 per capsule; logits; log_softmax
            e2s = small.tile([128, 8], F32, tag="s2")
            nc.vector.reduce_sum(
                e2s[:], e2[:, :J * 8].rearrange("p (j k) -> p k j", k=8), axis=AX.X)
            nc.vector.tensor_scalar_add(e2s[:], e2s[:], EPS)
            rS2 = small.tile([128, 8], F32, tag="rs2")
            nc.vector.reciprocal(rS2[:], e2s[:])
            usc = slabs.tile([128, 512], F32, tag="usc")
            uv = usc[:].rearrange("p (d k) -> p k d", k=8)
            pv = ups[:].rearrange("p (d k) -> p k d", k=8)
            for k in range(8):
                nc.scalar.activation(uv[:, k, :], pv[:, k, :], ACTF.Relu,
                                     scale=rS2[:, k:k + 1])
            lg = psum_l.tile([128, 40], F32, tag="logits")
            uts = []
            for ch in range(4):
                tp = psum_t.tile([128, 128], F32, tag="tpos")
                nc.tensor.transpose(tp[:], usc[:, ch * 128:(ch + 1) * 128], ident[:])
                ut = slabs.tile([128, 128], F32, tag=f"ut{ch}")
                nc.scalar.activation(ut[:], tp[:], ACTF.Copy)
                uts.append(ut)
            for ch in range(4):
                nc.tensor.matmul(lg[:], uts[ch][:], mlp_w_sb[ch][:],
                                 start=(ch == 0), stop=False)
            nc.tensor.matmul(lg[:], ones1[:], mlp_b_sb[:], start=False, stop=True)
            mx = small.tile([128, 1], F32, tag="mx")
            nc.vector.reduce_max(mx[:], lg[:], axis=AX.X)
            nc.vector.tensor_scalar_mul(mx[:], mx[:], -1.0)
            ex = small.tile([128, 40], F32, tag="ex")
            se = small.tile([128, 1], F32, tag="se")
            nc.scalar.activation(ex[:], lg[:], ACTF.Exp, bias=mx[:, 0:1],
                                 accum_out=se[:])
            lse = small.tile([128, 1], F32, tag="lse")
            nc.scalar.activation(lse[:], se[:], ACTF.Ln)
            ob = small.tile([128, 40], F32, tag="ob")
            nc.vector.tensor_scalar(ob[:], lg[:], mx[:, 0:1], lse[:, 0:1],
                                    op0=ALU.add, op1=ALU.subtract)
            nc.sync.dma_start(out_d[tl * 128:(tl + 1) * 128, :], ob[:])

    def weighted_scatter(self, tl, zp, e2p, identb, slabs, ups, J, c0):
        """u_psum[t, :] = sum_j e2p[t, j, k] * Zp[t, j, :] via PE identity-matmul."""
        nc = self.nc
        for (j0, jc) in chunks_of(J):
            w = slabs.tile([128, JC * 512], BF16, tag="w")
            nc.vector.tensor_mul(
                w[:, :jc * 512].rearrange("p (j d k) -> p j d k", d=64, k=8),
                zp[:, j0 * 512:(j0 + jc) * 512].rearrange(
                    "p (j d k) -> p j d k", d=64, k=8),
                e2p[:, j0 * 8:(j0 + jc) * 8].rearrange(
                    "p (j k) -> p j k", k=8).unsqueeze(2).broadcast_to(
                        (128, jc, 64, 8)))
            for j in range(jc):
                nc.tensor.matmul(ups[:], identb[:], w[:, j * 512:(j + 1) * 512],
                                 start=(j0 + j == 0), stop=(j0 + j == J - 1))


# ----------------------------------------------------------------------------
# Entry point
# ----------------------------------------------------------------------------

_CACHE = {}


def _prepare(x_nb, ppr, pca_w, pca_b, mlp_w, mlp_b, row_idx, col_idx, x_idx):
    lay = build_layout(row_idx, col_idx, ppr)
    has_pca_b = bool(np.any(pca_b))
    nc = build_program(lay, has_pca_b)
    in_maps = []
    # (d,k)-interleaved feature order: new index d*K+k <- old index k*DD+d.
    perm = (np.arange(K)[None, :] * DD + np.arange(DD)[:, None]).reshape(-1)
    shared = {
        "pca_w": np.ascontiguousarray(pca_w[:, perm]).astype(bf16),
        "mlp_w": np.ascontiguousarray(mlp_w[perm, :]).astype(np.float32),
        "mlp_b": np.ascontiguousarray(mlp_b).reshape(1, 40).astype(np.float32),
        "ident": np.eye(128, dtype=np.float32),
        "identb": np.eye(128).astype(bf16),
        "ones1": np.ones((1, 128), dtype=np.float32),
    }
    if has_pca_b:
        shared["pca_b"] = np.ascontiguousarray(pca_b.reshape(-1)[perm]).reshape(1, 512).astype(bf16)
        shared["ones1b"] = np.ones((1, 128), dtype=bf16)
    for c in range(NCORES):
        m = dict(shared)
        m.update(build_core_inputs(lay, c, x_nb, col_idx, ppr))
        in_maps.append(m)
    return lay, nc, in_maps


def _assemble(lay, results):
    out = np.empty((T, 40), dtype=np.float32)
    for c in range(NCORES):
        order = lay.cores[c]["order"]
        out[c * TPC + order] = results[c]["out"]
    return out


def kernel(**inputs):
    inputs = {k: np.asarray(v) for k, v in inputs.items()}
    lay, nc, in_maps = _prepare(**inputs)
    res = run_bass_kernel_spmd(nc, in_maps, list(range(NCORES)))
    return _assemble(lay, res.results)


# -- timing helper for test.py (not used by the grading harness) --------------

def bench(iters=10, **inputs):
    """Returns (output, best_ns) using a persistent jitted executable."""
    import jax
    from jax.sharding import Mesh, PartitionSpec
    from jax.experimental.shard_map import shard_map
    from concourse import bass2jax

    inputs = {k: np.asarray(v) for k, v in inputs.items()}
    lay, nc, in_maps = _prepare(**inputs)

    bass2jax.install_neuronx_cc_hook()
    partition_name = (nc.partition_id_tensor.name
                      if nc.partition_id_tensor else None)
    in_names, out_names, out_avals, zero_outs = [], [], [], []
    for alloc in nc.m.functions[0].allocations:
        if not isinstance(alloc, mybir.MemoryLocationSet):
            continue
        name = alloc.memorylocations[0].name
        if alloc.kind == "ExternalInput":
            if name != partition_name:
                in_names.append(name)
        elif alloc.kind == "ExternalOutput":
            out_names.append(name)
            shape = tuple(alloc.tensor_shape)
            dtype = mybir.dt.np(alloc.dtype)
            out_avals.append(jax.core.ShapedArray(shape, dtype))
            zero_outs.append(np.zeros(shape, dtype))
    n_params = len(in_names)
    n_outs = len(out_avals)
    all_names = list(in_names) + list(out_names)
    if partition_name is not None:
        all_names.append(partition_name)

    def _body(*args):
        operands = list(args)
        if partition_name is not None:
            operands.append(bass2jax.partition_id_tensor())
        outs = bass2jax._bass_exec_p.bind(
            *operands, out_avals=tuple(out_avals), in_names=tuple(all_names),
            out_names=tuple(out_names), lowering_input_output_aliases=(),
            sim_require_finite=True, sim_require_nnan=True, nc=nc)
        return tuple(outs)

    devices = jax.devices()[:NCORES]
    mesh = Mesh(np.asarray(devices), ("core",))
    donate = tuple(range(n_params, n_params + n_outs))
    sharded = jax.jit(
        shard_map(_body, mesh=mesh,
                  in_specs=(PartitionSpec("core"),) * (n_params + n_outs),
                  out_specs=(PartitionSpec("core"),) * n_outs,
                  check_rep=False),
        donate_argnums=donate, keep_unused=True)

    concat_in = [
        np.concatenate([np.asarray(in_maps[c][nm]) for c in range(NCORES)], axis=0)
        for nm in in_names]
    dev_in = [jax.device_put(a) for a in concat_in]

    def zeros():
        return [jax.device_put(np.zeros((NCORES * z.shape[0], *z.shape[1:]),
                                        z.dtype)) for z in zero_outs]

    out_arrs = sharded(*dev_in, *zeros())          # warmup + correctness
    jax.block_until_ready(out_arrs)
    results = [
        {nm: np.asarray(out_arrs[i]).reshape(NCORES, *out_avals[i].shape)[c]
         for i, nm in enumerate(out_names)}
        for c in range(NCORES)]
    output = _assemble(lay, results)

    best = float("inf")
    for _ in range(iters):
        zs = zeros()
        jax.block_until_ready(zs)
        t0 = time.perf_counter()
        o = sharded(*dev_in, *zs)
        jax.block_until_ready(o)
        best = min(best, time.perf_counter() - t0)
    return output, int(best * 1e9)


if __name__ == "__main__":
    import reference
    ins = {k: np.asarray(v) for k, v in reference.setup_inputs().items()}
    out = kernel(**ins)
    exp = np.asarray(reference.reference(**ins))
    err = np.abs(out - exp).max()
    print("max abs err:", err, "absmax:", np.abs(exp).max())
